# revision 4
# baseline (speedup 1.0000x reference)
"""GCNConv Trainium2 kernel.

Reference computation (all raw row-major reshapes):
    x_flat = x.reshape(-1, 64)                 # [960000, 64]
    h = (x_flat @ W).reshape(5000, 12288)
    agg = F @ h                                # [5000,5000] @ [5000,12288]
    out = agg.reshape(-1, 64) + bias           # [960000, 64]

Sharding: the 12288-wide feature axis splits into 8 shards of 1536 columns,
one per NeuronCore; F and W are replicated, so there are no collectives.
Because 1536 is a multiple of 64, shard c's columns of h depend only on
x.reshape(5000, 12288)[:, c*1536:(c+1)*1536] and W:
    h[:, c*1536 + g*64 + j] = sum_m X_c[v, g*64+m] * W[m, j]
i.e. per 64-column group g it is X_c[:, g-block] @ W.

Per-core device kernel (fp16 operands, fp32 PSUM accumulation):
  phase 1: Y_c = blockwise X_c @ W via PE transposes of [128,128] column
           pairs and K=128 matmuls against blockdiag(W, W); Y_c stays
           SBUF-resident as [128, 40, 1536] (vertex dim on partitions).
  phase 2: agg_c = F @ Y_c with F^T (host-pretransposed, zero-padded to
           5120) streamed as the stationary operand, 512-wide PSUM chunks
           accumulated over the 40 contraction tiles, bias added on the
           vector engine, fp32 rows DMA'd out.
"""

import numpy as np

import concourse.bass as bass
import concourse.mybir as mybir
import concourse.tile as tile
from concourse import bacc
from concourse.bass_utils import run_bass_kernel_spmd
from concourse.masks import make_identity

N_CORES = 8
NV = 5000            # vertex count
P = 128
NVP = 5120           # NV padded to a multiple of 128
KT = NVP // P        # 40 contraction / output row tiles
COLS_TOTAL = 12288   # B*T*c_out columns of the transformed feature matrix
COLS = COLS_TOTAL // N_CORES   # 1536 per core
CIN = 64
COUT = 64
GP = COLS // P       # 12 column-pair groups (two 64-blocks each)
FREE = 512           # matmul moving free dim (one fp32 PSUM bank)
NB = COLS // FREE    # 3 chunks
M_GRP = 2            # output row tiles per F^T staging DMA

MM_DT = mybir.dt.float16
MM_NP = np.float16


def build_nc():
    nc = bacc.Bacc(None, target_bir_lowering=False)

    xk = nc.dram_tensor("xk", [NVP, COLS], MM_DT, kind="ExternalInput")
    ft = nc.dram_tensor("ft", [NVP, NVP], MM_DT, kind="ExternalInput")
    w2 = nc.dram_tensor("w2", [P, P], MM_DT, kind="ExternalInput")
    biasb = nc.dram_tensor("biasb", [P, COLS], mybir.dt.float32, kind="ExternalInput")
    out = nc.dram_tensor("out", [NV, COLS], mybir.dt.float32, kind="ExternalOutput")

    xr = xk.rearrange("(it p) c -> it p c", p=P)      # [40, 128, 1536]
    ftr = ft.rearrange("(kt p) m -> kt p m", p=P)     # [40, 128, 5120]

    with tile.TileContext(nc) as tc:
        with (
            tc.tile_pool(name="const", bufs=1) as const,
            tc.tile_pool(name="ycache", bufs=1) as ypool,
        ):
            w2_sb = const.tile([P, P], MM_DT)
            nc.sync.dma_start(w2_sb[:], w2[:])
            bias_sb = const.tile([P, COLS], mybir.dt.float32)
            nc.sync.dma_start(bias_sb[:], biasb[:])
            ident = const.tile([P, P], MM_DT)
            make_identity(nc, ident)

            yc = ypool.tile([P, KT, COLS], MM_DT)

            # ---- phase 1: yc[:, it, :] = X tile @ blockdiag(W, W) ----
            with (
                tc.tile_pool(name="xin", bufs=3) as xin,
                tc.tile_pool(name="xtr", bufs=4) as xtr,
                tc.tile_pool(name="tpsum", bufs=2, space="PSUM") as tpsum,
                tc.tile_pool(name="ypsum", bufs=2, space="PSUM") as ypsum,
            ):
                for it in range(KT):
                    xt = xin.tile([P, COLS], MM_DT)
                    nc.sync.dma_start(xt[:], xr[it])
                    for g in range(GP):
                        pst = tpsum.tile([P, P], MM_DT)
                        nc.tensor.transpose(
                            pst[:], xt[:, g * P : (g + 1) * P], ident[:]
                        )
                        xgt = xtr.tile([P, P], MM_DT)
                        nc.any.tensor_copy(xgt[:], pst[:])
                        psy = ypsum.tile([P, P], mybir.dt.float32)
                        nc.tensor.matmul(
                            psy[:], xgt[:], w2_sb[:], start=True, stop=True
                        )
                        nc.any.tensor_copy(yc[:, it, g * P : (g + 1) * P], psy[:])

            # ---- phase 2: out rows = F @ Y + bias ----
            with (
                tc.tile_pool(name="fts", bufs=2) as ftsp,
                tc.tile_pool(name="osb", bufs=2) as osbp,
                tc.tile_pool(name="opsum", bufs=2, space="PSUM") as opsum,
            ):
                for mg in range(KT // M_GRP):
                    fts = ftsp.tile([P, KT, M_GRP * P], MM_DT)
                    nc.sync.dma_start(
                        fts[:],
                        ftr[:, :, mg * M_GRP * P : (mg + 1) * M_GRP * P].rearrange(
                            "kt p m -> p kt m"
                        ),
                    )
                    for ms in range(M_GRP):
                        m = mg * M_GRP + ms
                        psums = [
                            opsum.tile([P, FREE], mybir.dt.float32, name=f"ops{nb}")
                            for nb in range(NB)
                        ]
                        for kt in range(KT):
                            for nb in range(NB):
                                nc.tensor.matmul(
                                    psums[nb][:],
                                    fts[:, kt, ms * P : (ms + 1) * P],
                                    yc[:, kt, nb * FREE : (nb + 1) * FREE],
                                    start=(kt == 0),
                                    stop=(kt == KT - 1),
                                )
                        osb = osbp.tile([P, COLS], mybir.dt.float32)
                        for nb in range(NB):
                            nc.vector.tensor_add(
                                osb[:, nb * FREE : (nb + 1) * FREE],
                                psums[nb][:],
                                bias_sb[:, nb * FREE : (nb + 1) * FREE],
                            )
                        rows = min(P, NV - m * P)
                        if rows > 0:
                            nc.sync.dma_start(
                                out[m * P : m * P + rows, :], osb[:rows, :]
                            )

    nc.compile()
    return nc


def prepare_in_maps(x, gcnconv_filter, weight, bias):
    x2 = np.ascontiguousarray(x, dtype=np.float32).reshape(NV, COLS_TOTAL)

    ftp = np.zeros((NVP, NVP), dtype=MM_NP)
    ftp[:NV, :NV] = np.asarray(gcnconv_filter, dtype=np.float32).T

    w2 = np.zeros((P, P), dtype=MM_NP)
    w = np.asarray(weight, dtype=np.float32)
    w2[:CIN, :COUT] = w
    w2[CIN:, COUT:] = w

    bias_t = np.tile(np.asarray(bias, dtype=np.float32), COLS // COUT)
    biasb = np.ascontiguousarray(
        np.broadcast_to(bias_t[None, :], (P, COLS)), dtype=np.float32
    )

    in_maps = []
    for c in range(N_CORES):
        xc = np.zeros((NVP, COLS), dtype=MM_NP)
        xc[:NV, :] = x2[:, c * COLS : (c + 1) * COLS]
        in_maps.append({"xk": xc, "ft": ftp, "w2": w2, "biasb": biasb})
    return in_maps


def assemble_output(results):
    out2 = np.empty((NV, COLS_TOTAL), dtype=np.float32)
    for c in range(N_CORES):
        out2[:, c * COLS : (c + 1) * COLS] = results[c]["out"]
    return out2.reshape(NV * COLS_TOTAL // COUT, COUT)


_NC_CACHE = None


def kernel(x, gcnconv_filter, weight, bias):
    global _NC_CACHE
    if _NC_CACHE is None:
        _NC_CACHE = build_nc()
    in_maps = prepare_in_maps(x, gcnconv_filter, weight, bias)
    res = run_bass_kernel_spmd(_NC_CACHE, in_maps, core_ids=list(range(N_CORES)))
    return assemble_output(res.results)
